# revision 15
# baseline (speedup 1.0000x reference)
"""3-level db4 DWT (circular, stride-2) over x:(32, 8192, 64) on 8 TRN2 NeuronCores.

Strategy: data-parallel over batch (4 batches/core). Per core, the (seq, b*c=256)
activation matrix is processed with banded matmuls on the TensorEngine:
each 128-output chunk of A (lowpass) / D (highpass) coefficients is produced by
3 accumulating matmuls (even-tile taps, odd-tile taps, next-tile boundary taps)
against three 128x128 stationary band matrices. Sequence tiles of 128 are split
even/odd so a psum tile (128, 512) covers two consecutive output chunks with
N=512 moving operands. A-chunks are copied (cast) back to SBUF as the next
level's input; D-chunks (and final A3) stage through SBUF and DMA to DRAM.

Self-contained: hardcodes shapes/filters; needs only numpy + concourse on the
python path (axon TRN2 environment).
"""

import numpy as np

import concourse.bass as bass
import concourse.mybir as mybir
from concourse import bacc
from concourse.tile import TileContext
from concourse.bass_utils import run_bass_kernel_spmd  # noqa: F401 (fallback path)
from concourse import bass2jax

# db4 decomposition filters (pywt convention) — matches the reference.
DEC_LO = np.array([-0.010597401784997278, 0.032883011666982945, 0.030841381835986965,
                   -0.18703481171888114, -0.02798376941698385, 0.6308807679295904,
                   0.7148465705525415, 0.23037781330885523], dtype=np.float32)
DEC_HI = np.array([-0.23037781330885523, 0.7148465705525415, -0.6308807679295904,
                   -0.02798376941698385, 0.18703481171888114, 0.030841381835986965,
                   0.032883011666982945, -0.010597401784997278], dtype=np.float32)

N_CORES = 8
B = 32            # total batch
BC = B // N_CORES  # batches per core
C = 64            # channels
CB = BC * C       # 256 moving columns per core
N0 = 8192         # level-1 sequence length

# mm dtype: "float16" (1 cyc/row on PE, 11-bit mantissa — best speed/accuracy
# here since data is N(0,1)), "float32r" (fp32 bits, but ~2x slower: fused
# 4-byte weight reload per matmul), or "bfloat16".
MM_DTYPE = "float16"

# levels: (n_units, n_even_main_blocks) ; n_units = seq/512
LEVELS = [(16, 16), (8, 8), (4, 4)]


def _np_dt(name):
    if name == "bfloat16":
        import ml_dtypes
        return ml_dtypes.bfloat16
    if name == "float16":
        return np.float16
    return np.float32


def make_stationaries():
    """(128, 768) concat of S_AE,S_AO,S_AB,S_DE,S_DO,S_DB; S[s,m] = weight of
    moving row s for output partition m."""
    mats = [np.zeros((128, 128), np.float32) for _ in range(6)]
    S_AE, S_AO, S_AB, S_DE, S_DO, S_DB = mats
    for m in range(64):              # chunk low half: mains from even tile
        for j in range(8):
            s = 2 * m + j
            if s < 128:
                S_AE[s, m] = DEC_LO[j]
                S_DE[s, m] = DEC_HI[j]
            else:                    # spill into odd tile rows 0..5
                S_AO[s - 128, m] = DEC_LO[j]
                S_DO[s - 128, m] = DEC_HI[j]
    for m in range(64, 128):         # high half: mains from odd tile
        for j in range(8):
            s = 2 * m + j - 128
            if s < 128:
                S_AO[s, m] = DEC_LO[j]
                S_DO[s, m] = DEC_HI[j]
            else:                    # spill into next even tile rows 0..5
                S_AB[s - 128, m] = DEC_LO[j]
                S_DB[s - 128, m] = DEC_HI[j]
    return np.concatenate(mats, axis=1)


def build_bass(repeat=1, hw_loop=0):
    DT = getattr(mybir.dt, MM_DTYPE)
    f32 = mybir.dt.float32
    nc = bacc.Bacc(trn_type="TRN2", target_bir_lowering=False, num_devices=N_CORES)

    xe_d = nc.dram_tensor("xe", [128, 33 * CB], DT, kind="ExternalInput")
    xo_d = nc.dram_tensor("xo", [128, 32 * CB], DT, kind="ExternalInput")
    sm_d = nc.dram_tensor("smats", [128, 6 * 128], DT, kind="ExternalInput")
    d1_d = nc.dram_tensor("d1", [128, 32 * CB], f32, kind="ExternalOutput")
    d2_d = nc.dram_tensor("d2", [128, 16 * CB], f32, kind="ExternalOutput")
    d3_d = nc.dram_tensor("d3", [128, 8 * CB], f32, kind="ExternalOutput")
    a3_d = nc.dram_tensor("a3", [128, 8 * CB], f32, kind="ExternalOutput")

    with TileContext(nc) as tc:
        with (
            tc.tile_pool(name="const", bufs=1) as cpool,
            tc.tile_pool(name="data", bufs=1) as dpool,
            tc.tile_pool(name="stage", bufs=6) as spool,
            tc.tile_pool(name="psum", bufs=4, space="PSUM") as ppool,
        ):
            S = cpool.tile([128, 6 * 128], DT)
            nc.sync.dma_start(out=S, in_=sm_d[:])
            xe1 = dpool.tile([128, 33 * CB], DT)
            nc.sync.dma_start(out=xe1, in_=xe_d[:])
            xo1 = dpool.tile([128, 32 * CB], DT)
            nc.sync.dma_start(out=xo1, in_=xo_d[:])
            a1e = dpool.tile([128, 17 * CB], DT)
            a1o = dpool.tile([128, 16 * CB], DT)
            a2e = dpool.tile([128, 9 * CB], DT)
            a2o = dpool.tile([128, 8 * CB], DT)

            def do_level(in_e, in_o, n_units, a_dst, d_dram, a_dram=None):
                for U in range(n_units):
                    c0 = 2 * CB * U
                    psA = ppool.tile([128, 2 * CB], f32, tag="psA")
                    psD = ppool.tile([128, 2 * CB], f32, tag="psD")
                    mv_e = in_e[:, c0:c0 + 2 * CB]
                    mv_o = in_o[:, c0:c0 + 2 * CB]
                    mv_b = in_e[:, c0 + CB:c0 + 3 * CB]
                    nc.tensor.matmul(psA, S[:, 0:128], mv_e, start=True, stop=False)
                    nc.tensor.matmul(psA, S[:, 128:256], mv_o, start=False, stop=False)
                    nc.tensor.matmul(psA, S[:, 256:384], mv_b, start=False, stop=True)
                    nc.tensor.matmul(psD, S[:, 384:512], mv_e, start=True, stop=False)
                    nc.tensor.matmul(psD, S[:, 512:640], mv_o, start=False, stop=False)
                    nc.tensor.matmul(psD, S[:, 640:768], mv_b, start=False, stop=True)
                    if a_dst is not None:
                        a_e, a_o, n_main = a_dst
                        nc.vector.tensor_copy(out=a_e[:, CB * U:CB * (U + 1)],
                                              in_=psA[:, 0:CB])
                        nc.vector.tensor_copy(out=a_o[:, CB * U:CB * (U + 1)],
                                              in_=psA[:, CB:2 * CB])
                        if U == 0:  # circular wrap pad block
                            nc.vector.tensor_copy(
                                out=a_e[:, CB * n_main:CB * (n_main + 1)],
                                in_=psA[:, 0:CB])
                    else:
                        ast = spool.tile([128, 2 * CB], f32, tag="astage")
                        nc.vector.tensor_copy(out=ast, in_=psA[:])
                        nc.sync.dma_start(out=a_dram[:, c0:c0 + 2 * CB], in_=ast)
                    dst = spool.tile([128, 2 * CB], f32, tag="dstage")
                    nc.scalar.copy(out=dst, in_=psD[:])
                    nc.sync.dma_start(out=d_dram[:, c0:c0 + 2 * CB], in_=dst)

            def whole():
                do_level(xe1, xo1, LEVELS[0][0], (a1e, a1o, LEVELS[0][1]), d1_d)
                do_level(a1e, a1o, LEVELS[1][0], (a2e, a2o, LEVELS[1][1]), d2_d)
                do_level(a2e, a2o, LEVELS[2][0], None, d3_d, a3_d)

            if hw_loop:
                with tc.For_i(0, hw_loop, 1):
                    whole()
            else:
                for _ in range(repeat):
                    whole()

    nc.compile()
    return nc


_BUILD_CACHE = {}


def _get_runner(repeat=1, hw_loop=0):
    """Build (once) and return a jitted SPMD runner: fn(in_maps) -> results.

    Mirrors bass2jax.run_bass_via_pjrt's multi-core branch but caches the
    jitted callable so repeat calls don't re-trace/re-compile.
    """
    key = (MM_DTYPE, repeat, hw_loop)
    if key in _BUILD_CACHE:
        return _BUILD_CACHE[key]

    import jax
    from jax.sharding import Mesh, PartitionSpec
    from jax.experimental.shard_map import shard_map

    nc = build_bass(repeat, hw_loop)
    bass2jax.install_neuronx_cc_hook()

    partition_name = nc.partition_id_tensor.name if nc.partition_id_tensor else None
    in_names, out_names, out_avals, zero_outs = [], [], [], []
    for alloc in nc.m.functions[0].allocations:
        if not isinstance(alloc, mybir.MemoryLocationSet):
            continue
        name = alloc.memorylocations[0].name
        if alloc.kind == "ExternalInput":
            if name != partition_name:
                in_names.append(name)
        elif alloc.kind == "ExternalOutput":
            out_names.append(name)
            shape = tuple(alloc.tensor_shape)
            dtype = mybir.dt.np(alloc.dtype)
            out_avals.append(jax.core.ShapedArray(shape, dtype))
            zero_outs.append(np.zeros(shape, dtype))
    n_params = len(in_names)
    n_outs = len(out_avals)
    all_in_names = list(in_names) + list(out_names)
    if partition_name is not None:
        all_in_names.append(partition_name)
    donate = tuple(range(n_params, n_params + n_outs))

    def _body(*args):
        operands = list(args)
        if partition_name is not None:
            operands.append(bass2jax.partition_id_tensor())
        outs = bass2jax._bass_exec_p.bind(
            *operands,
            out_avals=tuple(out_avals),
            in_names=tuple(all_in_names),
            out_names=tuple(out_names),
            lowering_input_output_aliases=(),
            sim_require_finite=True,
            sim_require_nnan=True,
            nc=nc,
        )
        return tuple(outs)

    devices = jax.devices()[:N_CORES]
    mesh = Mesh(np.asarray(devices), ("core",))
    in_specs = (PartitionSpec("core"),) * (n_params + n_outs)
    out_specs = (PartitionSpec("core"),) * len(out_names)
    sharded = jax.jit(
        shard_map(_body, mesh=mesh, in_specs=in_specs, out_specs=out_specs,
                  check_rep=False),
        donate_argnums=donate, keep_unused=True,
    )

    def run(in_maps, raw=False):
        per_core = [[np.asarray(m[name]) for name in in_names] for m in in_maps]
        concat_in = [np.concatenate([per_core[c][i] for c in range(N_CORES)], axis=0)
                     for i in range(n_params)]
        concat_zeros = [np.zeros((N_CORES * z.shape[0], *z.shape[1:]), z.dtype)
                        for z in zero_outs]
        out_arrs = sharded(*concat_in, *concat_zeros)
        if raw:
            return out_arrs
        return [
            {name: np.asarray(out_arrs[i]).reshape(N_CORES, *out_avals[i].shape)[c]
             for i, name in enumerate(out_names)}
            for c in range(N_CORES)
        ]

    run.sharded = sharded
    run.in_names = in_names
    run.out_names = out_names
    run.out_avals = out_avals
    run.zero_outs = zero_outs
    run.nc = nc
    _BUILD_CACHE[key] = run
    return run


def _prep_core(x2d, np_dt):
    """x2d (8192, CB) fp32 -> xe (128, 33*CB), xo (128, 32*CB) in np_dt."""
    t = x2d.reshape(64, 128, CB)
    ev = np.concatenate([t[0::2], t[0:1]], axis=0)      # 33 tiles (wrap pad)
    od = t[1::2]                                        # 32 tiles
    xe = np.ascontiguousarray(ev.transpose(1, 0, 2).reshape(128, 33 * CB)).astype(np_dt)
    xo = np.ascontiguousarray(od.transpose(1, 0, 2).reshape(128, 32 * CB)).astype(np_dt)
    return xe, xo


def _chunk2seq(arr):
    """(128, nchunks*CB) chunk-major -> (nchunks*128, BC, C) -> (BC, n, C)."""
    nch = arr.shape[1] // CB
    seq = arr.reshape(128, nch, CB).transpose(1, 0, 2).reshape(nch * 128, BC, C)
    return seq.transpose(1, 0, 2)


def kernel(x):
    x = np.asarray(x, dtype=np.float32)
    assert x.shape == (B, N0, C)
    np_dt = _np_dt(MM_DTYPE)
    smats = make_stationaries().astype(np_dt)

    in_maps = []
    for i in range(N_CORES):
        xc = x[BC * i:BC * (i + 1)]                       # (BC, 8192, C)
        x2d = xc.transpose(1, 0, 2).reshape(N0, CB)       # (seq, cb)
        xe, xo = _prep_core(x2d, np_dt)
        in_maps.append({"xe": xe, "xo": xo, "smats": smats})

    res = _get_runner()(in_maps)

    A3 = np.empty((B, N0 // 8, C), np.float32)
    D3 = np.empty((B, N0 // 8, C), np.float32)
    D2 = np.empty((B, N0 // 4, C), np.float32)
    D1 = np.empty((B, N0 // 2, C), np.float32)
    for i in range(N_CORES):
        sl = slice(BC * i, BC * (i + 1))
        A3[sl] = _chunk2seq(np.asarray(res[i]["a3"], np.float32))
        D3[sl] = _chunk2seq(np.asarray(res[i]["d3"], np.float32))
        D2[sl] = _chunk2seq(np.asarray(res[i]["d2"], np.float32))
        D1[sl] = _chunk2seq(np.asarray(res[i]["d1"], np.float32))
    return (A3, D3, D2, D1)


# revision 26
# speedup vs baseline: 1.1571x; 1.1571x over previous
"""3-level db4 DWT (circular, stride-2) over x:(32, 8192, 64) on 8 TRN2 NeuronCores.

Strategy: data-parallel over batch (4 batches/core). Per core, the (seq, b*c=256)
activation matrix is processed with banded matmuls on the TensorEngine:
each 128-output chunk of A (lowpass) / D (highpass) coefficients is produced by
3 accumulating matmuls (even-tile taps, odd-tile taps, next-tile boundary taps)
against three 128x128 stationary band matrices. Sequence tiles of 128 are split
even/odd so a psum tile (128, 512) covers two consecutive output chunks with
N=512 moving operands. A-chunks are copied (cast) back to SBUF as the next
level's input; D-chunks (and final A3) stage through SBUF and DMA to DRAM.

Self-contained: hardcodes shapes/filters; needs only numpy + concourse on the
python path (axon TRN2 environment).
"""

import numpy as np

import concourse.bass as bass
import concourse.mybir as mybir
from concourse import bacc
from concourse.tile import TileContext
from concourse.bass_utils import run_bass_kernel_spmd  # noqa: F401 (fallback path)
from concourse import bass2jax

# db4 decomposition filters (pywt convention) — matches the reference.
DEC_LO = np.array([-0.010597401784997278, 0.032883011666982945, 0.030841381835986965,
                   -0.18703481171888114, -0.02798376941698385, 0.6308807679295904,
                   0.7148465705525415, 0.23037781330885523], dtype=np.float32)
DEC_HI = np.array([-0.23037781330885523, 0.7148465705525415, -0.6308807679295904,
                   -0.02798376941698385, 0.18703481171888114, 0.030841381835986965,
                   0.032883011666982945, -0.010597401784997278], dtype=np.float32)

N_CORES = 8
B = 32            # total batch
BC = B // N_CORES  # batches per core
C = 64            # channels
CB = BC * C       # 256 moving columns per core
N0 = 8192         # level-1 sequence length

# mm dtype: "float16" (1 cyc/row on PE, 11-bit mantissa — best speed/accuracy
# here since data is N(0,1)), "float32r" (fp32 bits, but ~2x slower: fused
# 4-byte weight reload per matmul), or "bfloat16".
MM_DTYPE = "float16"

# levels: (n_units, n_even_main_blocks) ; n_units = seq/512
LEVELS = [(16, 16), (8, 8), (4, 4)]


def _np_dt(name):
    if name == "bfloat16":
        import ml_dtypes
        return ml_dtypes.bfloat16
    if name == "float16":
        return np.float16
    return np.float32


def make_stationaries():
    """(128, 768) concat of S_AE,S_AO,S_AB,S_DE,S_DO,S_DB; S[s,m] = weight of
    moving row s for output partition m."""
    mats = [np.zeros((128, 128), np.float32) for _ in range(6)]
    S_AE, S_AO, S_AB, S_DE, S_DO, S_DB = mats
    for m in range(64):              # chunk low half: mains from even tile
        for j in range(8):
            s = 2 * m + j
            if s < 128:
                S_AE[s, m] = DEC_LO[j]
                S_DE[s, m] = DEC_HI[j]
            else:                    # spill into odd tile rows 0..5
                S_AO[s - 128, m] = DEC_LO[j]
                S_DO[s - 128, m] = DEC_HI[j]
    for m in range(64, 128):         # high half: mains from odd tile
        for j in range(8):
            s = 2 * m + j - 128
            if s < 128:
                S_AO[s, m] = DEC_LO[j]
                S_DO[s, m] = DEC_HI[j]
            else:                    # spill into next even tile rows 0..5
                S_AB[s - 128, m] = DEC_LO[j]
                S_DB[s - 128, m] = DEC_HI[j]
    return np.concatenate(mats, axis=1)


def build_bass(repeat=1, hw_loop=0, ablate=()):
    """ablate: subset of {"mmA","mmD","cpA","cpD","dmaout"} to skip (for
    bottleneck analysis only — results wrong when nonempty)."""
    DT = getattr(mybir.dt, MM_DTYPE)
    f32 = mybir.dt.float32
    nc = bacc.Bacc(trn_type="TRN2", target_bir_lowering=False, num_devices=N_CORES)

    xe_d = nc.dram_tensor("xe", [128, 33 * CB], DT, kind="ExternalInput")
    xo_d = nc.dram_tensor("xo", [128, 32 * CB], DT, kind="ExternalInput")
    sm_d = nc.dram_tensor("smats", [128, 6 * 128], DT, kind="ExternalInput")
    d1_d = nc.dram_tensor("d1", [128, 32 * CB], DT, kind="ExternalOutput")
    d2_d = nc.dram_tensor("d2", [128, 16 * CB], DT, kind="ExternalOutput")
    d3_d = nc.dram_tensor("d3", [128, 8 * CB], DT, kind="ExternalOutput")
    a3_d = nc.dram_tensor("a3", [128, 8 * CB], DT, kind="ExternalOutput")

    with TileContext(nc) as tc:
        with (
            tc.tile_pool(name="const", bufs=1) as cpool,
            tc.tile_pool(name="data", bufs=1) as dpool,
            tc.tile_pool(name="stage", bufs=6) as spool,
            tc.tile_pool(name="psum", bufs=4, space="PSUM") as ppool,
        ):
            S = cpool.tile([128, 6 * 128], DT)
            nc.sync.dma_start(out=S, in_=sm_d[:])
            # Level-1 input arrives in per-section tiles so matmuls can start
            # as soon as the first section lands. Even sections carry a 256-col
            # overlap (re-DMA'd) for the boundary matmul's +CB shifted slice.
            USEC = 4                      # U-steps per section
            W = 2 * CB                    # 512
            n_sec = LEVELS[0][0] // USEC  # 4
            xe_secs, xo_secs = [], []
            for s in range(n_sec):
                xe_secs.append(dpool.tile([128, USEC * W + CB], DT,
                                          tag=f"xe_s{s}", name=f"xe_s{s}"))
                xo_secs.append(dpool.tile([128, USEC * W], DT,
                                          tag=f"xo_s{s}", name=f"xo_s{s}"))

            def dma_in():
                for s in range(n_sec):
                    c = USEC * W * s
                    nc.sync.dma_start(out=xe_secs[s],
                                      in_=xe_d[:, c:c + USEC * W + CB])
                    nc.sync.dma_start(out=xo_secs[s],
                                      in_=xo_d[:, c:c + USEC * W])
            a1e = dpool.tile([128, 17 * CB], DT)
            a1o = dpool.tile([128, 16 * CB], DT)
            a2e = dpool.tile([128, 9 * CB], DT)
            a2o = dpool.tile([128, 8 * CB], DT)

            def do_level(in_e, in_o, n_units, a_dst, d_dram, a_dram=None,
                         secs=None):
                for U in range(n_units):
                    c0 = 2 * CB * U
                    psA = ppool.tile([128, 2 * CB], f32, tag="psA")
                    psD = ppool.tile([128, 2 * CB], f32, tag="psD")
                    if secs is not None:
                        se, so = secs[U // USEC]
                        cl = W * (U % USEC)
                        mv_e = se[:, cl:cl + 2 * CB]
                        mv_o = so[:, cl:cl + 2 * CB]
                        mv_b = se[:, cl + CB:cl + 3 * CB]
                    else:
                        mv_e = in_e[:, c0:c0 + 2 * CB]
                        mv_o = in_o[:, c0:c0 + 2 * CB]
                        mv_b = in_e[:, c0 + CB:c0 + 3 * CB]
                    if "mmA" not in ablate:
                        nc.tensor.matmul(psA, S[:, 0:128], mv_e, start=True, stop=False)
                        nc.tensor.matmul(psA, S[:, 128:256], mv_o, start=False, stop=False)
                        nc.tensor.matmul(psA, S[:, 256:384], mv_b, start=False, stop=True)
                    if "mmD" not in ablate:
                        nc.tensor.matmul(psD, S[:, 384:512], mv_e, start=True, stop=False)
                        nc.tensor.matmul(psD, S[:, 512:640], mv_o, start=False, stop=False)
                        nc.tensor.matmul(psD, S[:, 640:768], mv_b, start=False, stop=True)
                    if "cpA" in ablate:
                        pass
                    elif a_dst is not None:
                        a_e, a_o, n_main = a_dst
                        nc.vector.tensor_copy(out=a_e[:, CB * U:CB * (U + 1)],
                                              in_=psA[:, 0:CB])
                        nc.vector.tensor_copy(out=a_o[:, CB * U:CB * (U + 1)],
                                              in_=psA[:, CB:2 * CB])
                        if U == 0:  # circular wrap pad block
                            nc.vector.tensor_copy(
                                out=a_e[:, CB * n_main:CB * (n_main + 1)],
                                in_=psA[:, 0:CB])
                    else:
                        ast = spool.tile([128, 2 * CB], DT, tag="astage")
                        nc.vector.tensor_copy(out=ast, in_=psA[:])
                        if "dmaout" not in ablate:
                            nc.sync.dma_start(out=a_dram[:, c0:c0 + 2 * CB], in_=ast)
                    if "cpD" not in ablate:
                        dst = spool.tile([128, 2 * CB], DT, tag="dstage")
                        nc.scalar.copy(out=dst, in_=psD[:])
                        if "dmaout" not in ablate:
                            nc.sync.dma_start(out=d_dram[:, c0:c0 + 2 * CB], in_=dst)

            def whole():
                dma_in()
                do_level(None, None, LEVELS[0][0], (a1e, a1o, LEVELS[0][1]),
                         d1_d, secs=list(zip(xe_secs, xo_secs)))
                do_level(a1e, a1o, LEVELS[1][0], (a2e, a2o, LEVELS[1][1]), d2_d)
                do_level(a2e, a2o, LEVELS[2][0], None, d3_d, a3_d)

            if hw_loop:
                with tc.For_i(0, hw_loop, 1):
                    whole()
            else:
                for _ in range(repeat):
                    whole()

    nc.compile()
    return nc


_BUILD_CACHE = {}


def _get_runner(repeat=1, hw_loop=0):
    """Build (once) and return a jitted SPMD runner: fn(in_maps) -> results.

    Mirrors bass2jax.run_bass_via_pjrt's multi-core branch but caches the
    jitted callable so repeat calls don't re-trace/re-compile.
    """
    key = (MM_DTYPE, repeat, hw_loop)
    if key in _BUILD_CACHE:
        return _BUILD_CACHE[key]

    import jax
    from jax.sharding import Mesh, PartitionSpec
    from jax.experimental.shard_map import shard_map

    nc = build_bass(repeat, hw_loop)
    bass2jax.install_neuronx_cc_hook()

    partition_name = nc.partition_id_tensor.name if nc.partition_id_tensor else None
    in_names, out_names, out_avals, zero_outs = [], [], [], []
    for alloc in nc.m.functions[0].allocations:
        if not isinstance(alloc, mybir.MemoryLocationSet):
            continue
        name = alloc.memorylocations[0].name
        if alloc.kind == "ExternalInput":
            if name != partition_name:
                in_names.append(name)
        elif alloc.kind == "ExternalOutput":
            out_names.append(name)
            shape = tuple(alloc.tensor_shape)
            dtype = mybir.dt.np(alloc.dtype)
            out_avals.append(jax.core.ShapedArray(shape, dtype))
            zero_outs.append(np.zeros(shape, dtype))
    n_params = len(in_names)
    n_outs = len(out_avals)
    all_in_names = list(in_names) + list(out_names)
    if partition_name is not None:
        all_in_names.append(partition_name)
    donate = tuple(range(n_params, n_params + n_outs))

    def _body(*args):
        operands = list(args)
        if partition_name is not None:
            operands.append(bass2jax.partition_id_tensor())
        outs = bass2jax._bass_exec_p.bind(
            *operands,
            out_avals=tuple(out_avals),
            in_names=tuple(all_in_names),
            out_names=tuple(out_names),
            lowering_input_output_aliases=(),
            sim_require_finite=True,
            sim_require_nnan=True,
            nc=nc,
        )
        return tuple(outs)

    devices = jax.devices()[:N_CORES]
    mesh = Mesh(np.asarray(devices), ("core",))
    in_specs = (PartitionSpec("core"),) * (n_params + n_outs)
    out_specs = (PartitionSpec("core"),) * len(out_names)
    sharded = jax.jit(
        shard_map(_body, mesh=mesh, in_specs=in_specs, out_specs=out_specs,
                  check_rep=False),
        donate_argnums=donate, keep_unused=True,
    )

    def run(in_maps, raw=False):
        per_core = [[np.asarray(m[name]) for name in in_names] for m in in_maps]
        concat_in = [np.concatenate([per_core[c][i] for c in range(N_CORES)], axis=0)
                     for i in range(n_params)]
        concat_zeros = [np.zeros((N_CORES * z.shape[0], *z.shape[1:]), z.dtype)
                        for z in zero_outs]
        out_arrs = sharded(*concat_in, *concat_zeros)
        if raw:
            return out_arrs
        return [
            {name: np.asarray(out_arrs[i]).reshape(N_CORES, *out_avals[i].shape)[c]
             for i, name in enumerate(out_names)}
            for c in range(N_CORES)
        ]

    run.sharded = sharded
    run.in_names = in_names
    run.out_names = out_names
    run.out_avals = out_avals
    run.zero_outs = zero_outs
    run.nc = nc
    _BUILD_CACHE[key] = run
    return run


def _prep_core(x2d, np_dt):
    """x2d (8192, CB) fp32 -> xe (128, 33*CB), xo (128, 32*CB) in np_dt."""
    t = x2d.reshape(64, 128, CB)
    ev = np.concatenate([t[0::2], t[0:1]], axis=0)      # 33 tiles (wrap pad)
    od = t[1::2]                                        # 32 tiles
    xe = np.ascontiguousarray(ev.transpose(1, 0, 2).reshape(128, 33 * CB)).astype(np_dt)
    xo = np.ascontiguousarray(od.transpose(1, 0, 2).reshape(128, 32 * CB)).astype(np_dt)
    return xe, xo


def _chunk2seq(arr):
    """(128, nchunks*CB) chunk-major -> (nchunks*128, BC, C) -> (BC, n, C)."""
    nch = arr.shape[1] // CB
    seq = arr.reshape(128, nch, CB).transpose(1, 0, 2).reshape(nch * 128, BC, C)
    return seq.transpose(1, 0, 2)


def kernel(x):
    x = np.asarray(x, dtype=np.float32)
    assert x.shape == (B, N0, C)
    np_dt = _np_dt(MM_DTYPE)
    smats = make_stationaries().astype(np_dt)

    in_maps = []
    for i in range(N_CORES):
        xc = x[BC * i:BC * (i + 1)]                       # (BC, 8192, C)
        x2d = xc.transpose(1, 0, 2).reshape(N0, CB)       # (seq, cb)
        xe, xo = _prep_core(x2d, np_dt)
        in_maps.append({"xe": xe, "xo": xo, "smats": smats})

    res = _get_runner()(in_maps)

    A3 = np.empty((B, N0 // 8, C), np.float32)
    D3 = np.empty((B, N0 // 8, C), np.float32)
    D2 = np.empty((B, N0 // 4, C), np.float32)
    D1 = np.empty((B, N0 // 2, C), np.float32)
    for i in range(N_CORES):
        sl = slice(BC * i, BC * (i + 1))
        A3[sl] = _chunk2seq(np.asarray(res[i]["a3"]).astype(np.float32))
        D3[sl] = _chunk2seq(np.asarray(res[i]["d3"]).astype(np.float32))
        D2[sl] = _chunk2seq(np.asarray(res[i]["d2"]).astype(np.float32))
        D1[sl] = _chunk2seq(np.asarray(res[i]["d1"]).astype(np.float32))
    return (A3, D3, D2, D1)
